# revision 2
# baseline (speedup 1.0000x reference)
"""Trainium2 Bass kernel for nn_CNNLR (CNN + quadratic-expansion + linear regression).

Math: out[n] = w0 + w1 . f[n] + f[n]^T U f[n], where f[n] (1664 = 26 pos x 64 ch)
are the conv features and U is the block-upper-triangular reshape of the second
order part of the 1.33M-wide reg weight.

Device strategy (8 cores, one uniform SPMD program):
  - Convs (cheap, PE matmuls via im2col) replicated on every core, full batch.
  - The quadratic partial products vT[t'] = sum_{t<t'} U[t,t'] f[n,t] are sharded
    by t'-chunks of 128 across cores: each core holds a zero-padded per-core U
    slice (identical shapes => identical program) and runs the same matmul loop.
  - Host does the final tiny dot (vT . f), the first-order term and constants.
"""

import sys

sys.path.insert(0, "/opt/trn_rl_repo")

import numpy as np

B = 128          # batch
L = 26           # positions
C1, C2 = 128, 64
K1, K2 = 7, 5
NPOS = 25
NFEAT = L * C2   # 1664
H = 1 + NFEAT + (C2 * C2) * (NPOS * (NPOS + 1) // 2)

NCORES = 8
CB = 16          # batch columns per matmul chunk (26*16 = 416 <= 512 psum bank)
NCHUNK = B // CB
NTC = 13         # t' chunks of 128 (= 2 positions each)
QSLOTS = 2       # t' chunk slots per core (13 chunks over 8 cores)

# core -> its (up to QSLOTS) t'-chunk ids; -1 = padding slot (zero U data)
ASSIGN = [[0, 1], [2, 3], [4, 5], [6, 7], [8, 9], [10, 11], [12, -1], [-1, -1]]

_CACHE: dict = {}


def _build_program():
    import concourse.mybir as mybir
    import concourse.tile as tile
    from concourse import bacc

    f32 = mybir.dt.float32
    nc = bacc.Bacc(
        "TRN2",
        target_bir_lowering=False,
        debug=False,
        enable_asserts=False,
        num_devices=NCORES,
    )

    X1 = nc.dram_tensor("x1_im2col", [K1 * 4, L, B], f32, kind="ExternalInput").ap()
    W1 = nc.dram_tensor("w1_lhsT", [K1 * 4, C1], f32, kind="ExternalInput").ap()
    B1 = nc.dram_tensor("b1_col", [C1, 1], f32, kind="ExternalInput").ap()
    W2 = nc.dram_tensor("w2_lhsT", [C1, K2 * C2], f32, kind="ExternalInput").ap()
    B2 = nc.dram_tensor("b2_col", [C2, 1], f32, kind="ExternalInput").ap()
    UQ = nc.dram_tensor("u_blocks", [C2, QSLOTS, L, 128], f32, kind="ExternalInput").ap()
    FT = nc.dram_tensor("feat_t", [C2, L, B], f32, kind="ExternalOutput").ap()
    VT = nc.dram_tensor("v_t", [128, QSLOTS, B], f32, kind="ExternalOutput").ap()

    Relu = mybir.ActivationFunctionType.Relu
    LP = L + 4  # conv2 needs pad-2 on both sides of the 26 positions

    with tile.TileContext(nc) as tc:
        with (
            tc.tile_pool(name="const", bufs=1) as cpool,
            tc.tile_pool(name="work", bufs=1) as wpool,
            tc.tile_pool(name="ps1", bufs=2, space="PSUM") as ps1,
            tc.tile_pool(name="ps2", bufs=2, space="PSUM") as ps2,
            tc.tile_pool(name="psv", bufs=2, space="PSUM") as psv,
        ):
            x1 = cpool.tile([K1 * 4, L, B], f32)
            w1 = cpool.tile([K1 * 4, C1], f32)
            b1 = cpool.tile([C1, 1], f32)
            w2 = cpool.tile([C1, K2 * C2], f32)
            b2 = cpool.tile([C2, 1], f32)
            uq = cpool.tile([C2, QSLOTS, L, 128], f32)

            h1 = wpool.tile([C1, LP, B], f32)      # conv1 out, padded positions
            ft = wpool.tile([C2, L, B], f32)       # conv2 out (features, featT)
            vts = wpool.tile([128, QSLOTS, B], f32)

            nc.sync.dma_start(w1[:], W1[:])
            nc.sync.dma_start(b1[:], B1[:])
            nc.sync.dma_start(w2[:], W2[:])
            nc.sync.dma_start(b2[:], B2[:])
            nc.sync.dma_start(x1[:], X1[:])
            nc.sync.dma_start(uq[:], UQ[:])

            # zero the conv2 halo columns of h1
            nc.gpsimd.memset(h1[:, 0:2, :], 0.0)
            nc.gpsimd.memset(h1[:, L + 2 : L + 4, :], 0.0)

            for c in range(NCHUNK):
                cs = slice(c * CB, (c + 1) * CB)
                y1 = ps1.tile([C1, L, CB], f32)
                nc.tensor.matmul(y1[:], w1[:], x1[:, :, cs], start=True, stop=True)
                nc.scalar.activation(h1[:, 2 : 2 + L, cs], y1[:], Relu, bias=b1[:])

                y2 = ps2.tile([C2, L, CB], f32)
                for t in range(K2):
                    nc.tensor.matmul(
                        y2[:],
                        w2[:, t * C2 : (t + 1) * C2],
                        h1[:, t : t + L, cs],
                        start=(t == 0),
                        stop=(t == K2 - 1),
                    )
                nc.scalar.activation(ft[:, :, cs], y2[:], Relu, bias=b2[:])

            nc.sync.dma_start(FT[:], ft[:])

            for q in range(QSLOTS):
                vp = psv.tile([128, B], f32)
                for i in range(L):
                    nc.tensor.matmul(
                        vp[:],
                        uq[:, q, i, :],
                        ft[:, i, :],
                        start=(i == 0),
                        stop=(i == L - 1),
                    )
                nc.scalar.copy(vts[:, q, :], vp[:])

            nc.sync.dma_start(VT[:], vts[:])

    nc.compile()
    return nc


def _get_program():
    if "nc" not in _CACHE:
        _CACHE["nc"] = _build_program()
    return _CACHE["nc"]


def _host_prep(x, conv1_w, conv1_b, conv2_w, conv2_b, reg_w):
    """Build per-core input maps (all numpy float32, layouts match the program)."""
    x = np.asarray(x)
    conv1_w = np.asarray(conv1_w, np.float32)
    conv1_b = np.asarray(conv1_b, np.float32)
    conv2_w = np.asarray(conv2_w, np.float32)
    conv2_b = np.asarray(conv2_b, np.float32)
    reg_w = np.asarray(reg_w, np.float32)

    # one-hot with pad-3 halo, positions live at columns 3..28
    ohp = np.zeros((B, L + 6, 4), np.float32)
    n_idx = np.repeat(np.arange(B), L)
    l_idx = np.tile(np.arange(L) + 3, B)
    ohp[n_idx, l_idx, x.reshape(-1).astype(np.int64)] = 1.0

    # conv1 im2col: X1[(t*4+c), l, n] = onehot[n, l+t-3, c]
    x1 = np.empty((K1, 4, L, B), np.float32)
    for t in range(K1):
        x1[t] = ohp[:, t : t + L, :].transpose(2, 1, 0)
    x1 = np.ascontiguousarray(x1.reshape(K1 * 4, L, B))

    w1 = np.ascontiguousarray(conv1_w.transpose(2, 1, 0).reshape(K1 * 4, C1))
    w2 = np.ascontiguousarray(conv2_w.transpose(1, 2, 0).reshape(C1, K2 * C2))
    b1 = np.ascontiguousarray(conv1_b.reshape(C1, 1))
    b2 = np.ascontiguousarray(conv2_b.reshape(C2, 1))

    # second-order weight blocks: blocks[i][j, p-(i+1), k] = U[i*64+j, p*64+k]
    w2nd = reg_w[0, 1 + NFEAT :]
    sizes = [(NPOS - i) * C2 * C2 for i in range(NPOS)]
    offs = np.concatenate([[0], np.cumsum(sizes)])
    blocks = [
        w2nd[offs[i] : offs[i + 1]].reshape(C2, NPOS - i, C2) for i in range(NPOS)
    ]

    uqs = np.zeros((NCORES, C2, QSLOTS, L, 128), np.float32)
    for core in range(NCORES):
        for q, a in enumerate(ASSIGN[core]):
            if a < 0:
                continue
            for p in (2 * a, 2 * a + 1):
                if p < 1 or p > NPOS:
                    continue
                r0 = (p - 2 * a) * C2
                for i in range(p):
                    uqs[core, :, q, i, r0 : r0 + C2] = blocks[i][:, p - i - 1, :]

    in_maps = []
    for core in range(NCORES):
        in_maps.append(
            {
                "x1_im2col": x1,
                "w1_lhsT": w1,
                "b1_col": b1,
                "w2_lhsT": w2,
                "b2_col": b2,
                "u_blocks": np.ascontiguousarray(uqs[core]),
            }
        )
    return in_maps


def _host_post(results, reg_w, reg_b):
    reg_w = np.asarray(reg_w, np.float32)
    reg_b = np.asarray(reg_b, np.float32)
    ft = results[0]["feat_t"]  # [C2, L, B]
    feat = ft.transpose(2, 1, 0).reshape(B, NFEAT).astype(np.float64)

    w1vec = reg_w[0, 1 : 1 + NFEAT].astype(np.float64)
    out = feat @ w1vec + np.float64(reg_w[0, 0]) + np.float64(reg_b[0])

    feat2 = feat.reshape(B, NTC, 128)
    for core in range(NCORES):
        vt = results[core]["v_t"].astype(np.float64)  # [128, QSLOTS, B]
        for q, a in enumerate(ASSIGN[core]):
            if a < 0:
                continue
            out += np.einsum("rn,nr->n", vt[:, q, :], feat2[:, a, :])
    return out.astype(np.float32)


def _install_ntff_shim():
    """Register the axon NTFF profile hook that the agent image's antenv lacks.

    Replicates trn_boot._ntff_profile_via_ctypes against /opt/axon/libaxon_pjrt.so
    and exposes it via a synthetic antenv.axon_hooks module so that
    bass_utils.run_bass_kernel_spmd(trace=True) can find it.
    """
    import sys as _sys
    import types

    if "antenv.axon_hooks" in _sys.modules:
        return
    _sys.path.insert(0, "/root/.axon_site/trn_agent_boot")
    try:
        import trn_boot
    finally:
        _sys.path.pop(0)
    hook = trn_boot._ntff_profile_via_ctypes("/opt/axon/libaxon_pjrt.so")
    mod = types.ModuleType("antenv.axon_hooks")
    mod._hook = hook
    mod.get_axon_ntff_profile_hook = lambda: mod._hook
    mod.set_axon_ntff_profile_hook = lambda h: setattr(mod, "_hook", h)
    _sys.modules["antenv.axon_hooks"] = mod
    import antenv

    antenv.axon_hooks = mod


def _run(inputs, trace=False):
    from concourse.bass_utils import run_bass_kernel_spmd

    if trace:
        _install_ntff_shim()
    nc = _get_program()
    in_maps = _host_prep(
        inputs["x"],
        inputs["conv1_w"],
        inputs["conv1_b"],
        inputs["conv2_w"],
        inputs["conv2_b"],
        inputs["reg_w"],
    )
    br = run_bass_kernel_spmd(nc, in_maps, core_ids=list(range(NCORES)), trace=trace)
    out = _host_post(br.results, inputs["reg_w"], inputs["reg_b"])
    return out, br


def kernel(**inputs) -> np.ndarray:
    out, _ = _run(inputs, trace=False)
    return out


# revision 6
# speedup vs baseline: 2.2013x; 2.2013x over previous
"""Trainium2 Bass kernel for nn_CNNLR (CNN + quadratic-expansion + linear regression).

Math: out[n] = w0 + w1 . f[n] + f[n]^T U f[n], where f[n] (1664 = 26 pos x 64 ch)
are the conv features and U is the block-upper-triangular reshape of the second
order part of the 1.33M-wide reg weight.

Device strategy (8 cores, one uniform SPMD program):
  - Convs (cheap, PE matmuls via im2col) replicated on every core, full batch.
  - The quadratic partial products vT[t'] = sum_{t<t'} U[t,t'] f[n,t] are sharded
    by t'-chunks of 128 across cores: each core holds a zero-padded per-core U
    slice (identical shapes => identical program) and runs the same matmul loop.
  - Host does the final tiny dot (vT . f), the first-order term and constants.
"""

import sys

sys.path.insert(0, "/opt/trn_rl_repo")

import numpy as np

B = 128          # batch
L = 26           # positions
C1, C2 = 128, 64
K1, K2 = 7, 5
NPOS = 25
NFEAT = L * C2   # 1664
H = 1 + NFEAT + (C2 * C2) * (NPOS * (NPOS + 1) // 2)

NCORES = 8
CB = 16          # batch columns per matmul chunk (26*16 = 416 <= 512 psum bank)
NCHUNK = B // CB
NTC = 13         # t' chunks of 128 (= 2 positions each)
QSLOTS = 2       # t' chunk slots per core (13 chunks over 8 cores)

# core -> its (up to QSLOTS) t'-chunk ids; -1 = padding slot (zero U data)
ASSIGN = [[0, 1], [2, 3], [4, 5], [6, 7], [8, 9], [10, 11], [12, -1], [-1, -1]]

import os

DTYPE = os.environ.get("BASS_KERNEL_DTYPE", "bf16")  # "bf16" or "fp32"

_CACHE: dict = {}


def _np_dt():
    import ml_dtypes

    return np.dtype(ml_dtypes.bfloat16) if DTYPE == "bf16" else np.dtype(np.float32)


def _build_program():
    import concourse.mybir as mybir
    import concourse.tile as tile
    from concourse import bacc

    f32 = mybir.dt.float32
    dt = mybir.dt.bfloat16 if DTYPE == "bf16" else mybir.dt.float32
    nc = bacc.Bacc(
        "TRN2",
        target_bir_lowering=False,
        debug=False,
        enable_asserts=False,
        num_devices=NCORES,
    )

    X1 = nc.dram_tensor("x1_im2col", [K1 * 4, L, B], dt, kind="ExternalInput").ap()
    W1 = nc.dram_tensor("w1_lhsT", [K1 * 4, C1], dt, kind="ExternalInput").ap()
    B1 = nc.dram_tensor("b1_col", [C1, 1], f32, kind="ExternalInput").ap()
    W2 = nc.dram_tensor("w2_lhsT", [C1, K2 * C2], dt, kind="ExternalInput").ap()
    B2 = nc.dram_tensor("b2_col", [C2, 1], f32, kind="ExternalInput").ap()
    UQ = nc.dram_tensor("u_blocks", [C2, QSLOTS, L, 128], dt, kind="ExternalInput").ap()
    FT = nc.dram_tensor("feat_t", [C2, L, B], dt, kind="ExternalOutput").ap()
    VT = nc.dram_tensor("v_t", [128, QSLOTS, B], f32, kind="ExternalOutput").ap()

    Relu = mybir.ActivationFunctionType.Relu
    LP = L + 4  # conv2 needs pad-2 on both sides of the 26 positions

    with tile.TileContext(nc) as tc:
        with (
            tc.tile_pool(name="const", bufs=1) as cpool,
            tc.tile_pool(name="work", bufs=1) as wpool,
            tc.tile_pool(name="ps1", bufs=2, space="PSUM") as ps1,
            tc.tile_pool(name="ps2", bufs=2, space="PSUM") as ps2,
            tc.tile_pool(name="psv", bufs=2, space="PSUM") as psv,
        ):
            x1 = cpool.tile([K1 * 4, L, B], dt)
            w1 = cpool.tile([K1 * 4, C1], dt)
            b1 = cpool.tile([C1, 1], f32)
            w2 = cpool.tile([C1, K2 * C2], dt)
            b2 = cpool.tile([C2, 1], f32)
            uq = cpool.tile([C2, QSLOTS, L, 128], dt)

            h1 = wpool.tile([C1, LP, B], dt)       # conv1 out, padded positions
            ft = wpool.tile([C2, L, B], dt)        # conv2 out (features, featT)
            vts = wpool.tile([128, QSLOTS, B], f32)

            nc.sync.dma_start(w1[:], W1[:])
            nc.sync.dma_start(b1[:], B1[:])
            nc.sync.dma_start(w2[:], W2[:])
            nc.sync.dma_start(b2[:], B2[:])
            nc.sync.dma_start(x1[:], X1[:])
            nc.sync.dma_start(uq[:], UQ[:])

            # zero the conv2 halo columns of h1
            nc.gpsimd.memset(h1[:, 0:2, :], 0.0)
            nc.gpsimd.memset(h1[:, L + 2 : L + 4, :], 0.0)

            for c in range(NCHUNK):
                cs = slice(c * CB, (c + 1) * CB)
                y1 = ps1.tile([C1, L, CB], f32)
                nc.tensor.matmul(y1[:], w1[:], x1[:, :, cs], start=True, stop=True)
                nc.scalar.activation(h1[:, 2 : 2 + L, cs], y1[:], Relu, bias=b1[:])

                y2 = ps2.tile([C2, L, CB], f32)
                for t in range(K2):
                    nc.tensor.matmul(
                        y2[:],
                        w2[:, t * C2 : (t + 1) * C2],
                        h1[:, t : t + L, cs],
                        start=(t == 0),
                        stop=(t == K2 - 1),
                    )
                nc.scalar.activation(ft[:, :, cs], y2[:], Relu, bias=b2[:])

            nc.sync.dma_start(FT[:], ft[:])

            for q in range(QSLOTS):
                vp = psv.tile([128, B], f32)
                for i in range(L):
                    nc.tensor.matmul(
                        vp[:],
                        uq[:, q, i, :],
                        ft[:, i, :],
                        start=(i == 0),
                        stop=(i == L - 1),
                    )
                nc.scalar.copy(vts[:, q, :], vp[:])

            nc.sync.dma_start(VT[:], vts[:])

    nc.compile()
    return nc


def _get_program():
    if "nc" not in _CACHE:
        _CACHE["nc"] = _build_program()
    return _CACHE["nc"]


def _host_prep(x, conv1_w, conv1_b, conv2_w, conv2_b, reg_w):
    """Build per-core input maps (all numpy float32, layouts match the program)."""
    x = np.asarray(x)
    conv1_w = np.asarray(conv1_w, np.float32)
    conv1_b = np.asarray(conv1_b, np.float32)
    conv2_w = np.asarray(conv2_w, np.float32)
    conv2_b = np.asarray(conv2_b, np.float32)
    reg_w = np.asarray(reg_w, np.float32)

    # one-hot with pad-3 halo, positions live at columns 3..28
    ohp = np.zeros((B, L + 6, 4), np.float32)
    n_idx = np.repeat(np.arange(B), L)
    l_idx = np.tile(np.arange(L) + 3, B)
    ohp[n_idx, l_idx, x.reshape(-1).astype(np.int64)] = 1.0

    # conv1 im2col: X1[(t*4+c), l, n] = onehot[n, l+t-3, c]
    x1 = np.empty((K1, 4, L, B), np.float32)
    for t in range(K1):
        x1[t] = ohp[:, t : t + L, :].transpose(2, 1, 0)
    x1 = np.ascontiguousarray(x1.reshape(K1 * 4, L, B))

    w1 = np.ascontiguousarray(conv1_w.transpose(2, 1, 0).reshape(K1 * 4, C1))
    w2 = np.ascontiguousarray(conv2_w.transpose(1, 2, 0).reshape(C1, K2 * C2))
    b1 = np.ascontiguousarray(conv1_b.reshape(C1, 1))
    b2 = np.ascontiguousarray(conv2_b.reshape(C2, 1))

    # second-order weight blocks: blocks[i][j, p-(i+1), k] = U[i*64+j, p*64+k]
    w2nd = reg_w[0, 1 + NFEAT :]
    sizes = [(NPOS - i) * C2 * C2 for i in range(NPOS)]
    offs = np.concatenate([[0], np.cumsum(sizes)])
    blocks = [
        w2nd[offs[i] : offs[i + 1]].reshape(C2, NPOS - i, C2) for i in range(NPOS)
    ]

    uqs = np.zeros((NCORES, C2, QSLOTS, L, 128), np.float32)
    for core in range(NCORES):
        for q, a in enumerate(ASSIGN[core]):
            if a < 0:
                continue
            for p in (2 * a, 2 * a + 1):
                if p < 1 or p > NPOS:
                    continue
                r0 = (p - 2 * a) * C2
                for i in range(p):
                    uqs[core, :, q, i, r0 : r0 + C2] = blocks[i][:, p - i - 1, :]

    wdt = _np_dt()
    in_maps = []
    for core in range(NCORES):
        in_maps.append(
            {
                "x1_im2col": x1.astype(wdt),
                "w1_lhsT": w1.astype(wdt),
                "b1_col": b1,
                "w2_lhsT": w2.astype(wdt),
                "b2_col": b2,
                "u_blocks": np.ascontiguousarray(uqs[core]).astype(wdt),
            }
        )
    return in_maps


def _host_post(results, reg_w, reg_b):
    reg_w = np.asarray(reg_w, np.float32)
    reg_b = np.asarray(reg_b, np.float32)
    ft = np.asarray(results[0]["feat_t"], np.float32)  # [C2, L, B]
    feat = ft.transpose(2, 1, 0).reshape(B, NFEAT).astype(np.float64)

    w1vec = reg_w[0, 1 : 1 + NFEAT].astype(np.float64)
    out = feat @ w1vec + np.float64(reg_w[0, 0]) + np.float64(reg_b[0])

    feat2 = feat.reshape(B, NTC, 128)
    for core in range(NCORES):
        vt = results[core]["v_t"].astype(np.float64)  # [128, QSLOTS, B]
        for q, a in enumerate(ASSIGN[core]):
            if a < 0:
                continue
            out += np.einsum("rn,nr->n", vt[:, q, :], feat2[:, a, :])
    return out.astype(np.float32)


def _install_ntff_shim():
    """Register the axon NTFF profile hook that the agent image's antenv lacks.

    Replicates trn_boot._ntff_profile_via_ctypes against /opt/axon/libaxon_pjrt.so
    and exposes it via a synthetic antenv.axon_hooks module so that
    bass_utils.run_bass_kernel_spmd(trace=True) can find it.
    """
    import sys as _sys
    import types

    if "antenv.axon_hooks" in _sys.modules:
        return
    _sys.path.insert(0, "/root/.axon_site/trn_agent_boot")
    try:
        import trn_boot
    finally:
        _sys.path.pop(0)
    hook = trn_boot._ntff_profile_via_ctypes("/opt/axon/libaxon_pjrt.so")
    mod = types.ModuleType("antenv.axon_hooks")
    mod._hook = hook
    mod.get_axon_ntff_profile_hook = lambda: mod._hook
    mod.set_axon_ntff_profile_hook = lambda h: setattr(mod, "_hook", h)
    _sys.modules["antenv.axon_hooks"] = mod
    import antenv

    antenv.axon_hooks = mod


def _run(inputs, trace=False):
    from concourse.bass_utils import run_bass_kernel_spmd

    if trace:
        _install_ntff_shim()
    nc = _get_program()
    in_maps = _host_prep(
        inputs["x"],
        inputs["conv1_w"],
        inputs["conv1_b"],
        inputs["conv2_w"],
        inputs["conv2_b"],
        inputs["reg_w"],
    )
    br = run_bass_kernel_spmd(nc, in_maps, core_ids=list(range(NCORES)), trace=trace)
    out = _host_post(br.results, inputs["reg_w"], inputs["reg_b"])
    return out, br


def kernel(**inputs) -> np.ndarray:
    out, _ = _run(inputs, trace=False)
    return out
